# revision 1
# baseline (speedup 1.0000x reference)
"""Trainium2 Bass kernel for the FNO-style spectral layer.

Math: reference computes y = irfft(rfft(x) + delta) along L where delta
only touches output bins 0..63:
    delta[k] = fre[index[k]] * wr[k] + i * fim[index[k]] * wi[k]
By linearity of rfft/irfft, y = x + x @ P @ Q where
    P[n, k]      =  wr[k] * cos(2*pi*index[k]*n/L) / sqrt(L)
    P[n, 64+k]   = -wi[k] * sin(2*pi*index[k]*n/L) / sqrt(L)
    Q[k, n]      =  c_k * cos(2*pi*k*n/L)          (c_0 = 1/sqrt(L), else 2/sqrt(L))
    Q[64+k, n]   = -c_k * sin(2*pi*k*n/L)
(the jax irfft ignores the imaginary part of bin 0; row 64 of Q is zero
anyway since sin(0) == 0).

Device kernel (pure data parallel over 8 cores, batch sharded; 11264
rows/core as 256-row edge tiles + 512-row body tiles): DMA x in natural
layout on the SP HWDGE ring, PE-transpose 125-column chunks into PSUM,
ACT-copy them to SBUF rounding to float32r, matmul A^T = P^T x^T (f32r
runs at full fp32 speed when the moving dim is >= 256), then
corr = A @ Q per 128-row sub-tile, add x + corr on DVE, and store y on
the ACT HWDGE ring (separate ring so stores never head-block loads).

Measured: ~256-260 us HW exec per 8-core run, rel err 1.4e-06 — at
98-100% of the pair-shared HBM roofline (each NeuronCore pair moves
182 MB through one 716 GB/s stack -> 254.6 us floor).
"""

import sys

if "/opt/trn_rl_repo" not in sys.path:
    sys.path.insert(0, "/opt/trn_rl_repo")

import numpy as np

import concourse.bass as bass  # noqa: F401  (kept for AP helpers)
import concourse.mybir as mybir
from concourse import bacc
from concourse.bass_utils import run_bass_kernel_spmd
from concourse.masks import make_identity
from concourse.tile import TileContext

B, E, L = 4096, 22, 1000
MODES = 64
NCORES = 8
ROWS = B * E                  # 90112
R_CORE = ROWS // NCORES       # 11264
SUP = 512                     # rows per super-tile
NSUB = SUP // 128             # 4
NSUP = R_CORE // SUP          # 22
KC = 125                      # contraction chunk (8 * 125 = 1000)
NCH = L // KC                 # 8
HALF = L // 2                 # 500

F32 = mybir.dt.float32
F32R = mybir.dt.float32r

# knobs (module-level so test.py can flip them before first kernel() call)
TRACE = False
MM_DTYPE = F32R               # matmul operand dtype (f32r fast fp32 mode)
LAST_RESULT = None


def _build_pq(fweights, fweights_im, index):
    """Host-side: analysis P [L, 2m] and synthesis Q [2m, L] in float32."""
    fw = np.asarray(fweights, dtype=np.float64)
    fwi = np.asarray(fweights_im, dtype=np.float64)
    idx = np.asarray(index, dtype=np.int64)
    m = idx.shape[0]
    widx = np.concatenate([[0], np.arange(1, m) + 1])
    wr = fw[widx, 0]
    wi = fwi[widx, 0]
    n = np.arange(L, dtype=np.float64)
    ang_in = 2.0 * np.pi * np.outer(n, idx.astype(np.float64)) / L
    P = np.zeros((L, 2 * m), dtype=np.float64)
    P[:, :m] = np.cos(ang_in) * wr / np.sqrt(L)
    P[:, m:] = -np.sin(ang_in) * wi / np.sqrt(L)
    k_out = np.arange(m, dtype=np.float64)
    ang_out = 2.0 * np.pi * np.outer(k_out, n) / L
    c = np.full(m, 2.0 / np.sqrt(L))
    c[0] = 1.0 / np.sqrt(L)
    Q = np.zeros((2 * m, L), dtype=np.float64)
    Q[:m, :] = np.cos(ang_out) * c[:, None]
    Q[m:, :] = -np.sin(ang_out) * c[:, None]
    return P.astype(np.float32), Q.astype(np.float32)


_nc_cache = None


def _build_bass():
    nc = bacc.Bacc(None, target_bir_lowering=False)
    x_d = nc.dram_tensor("x", [R_CORE, L], F32, kind="ExternalInput")
    p_d = nc.dram_tensor("p", [L, 2 * MODES], F32, kind="ExternalInput")
    q_d = nc.dram_tensor("q", [2 * MODES, L], F32, kind="ExternalInput")
    y_d = nc.dram_tensor("y", [R_CORE, L], F32, kind="ExternalOutput")

    RND = MM_DTYPE  # matmul operands must be rounded to this dtype

    with TileContext(nc) as tc:
        with (
            tc.tile_pool(name="consts", bufs=1) as consts,
            tc.tile_pool(name="xin", bufs=4) as xin,
            tc.tile_pool(name="xtp", bufs=3) as xtp,
            tc.tile_pool(name="apool", bufs=3) as apool,
            tc.tile_pool(name="yout", bufs=3) as yout,
            tc.tile_pool(name="ps_xt", bufs=4, space="PSUM") as ps_xt,
            tc.tile_pool(name="ps_a", bufs=2, space="PSUM") as ps_a,
            tc.tile_pool(name="ps_c", bufs=2, space="PSUM") as ps_c,
        ):
            ident = consts.tile([128, 128], F32)
            make_identity(nc, ident)
            # stage P/Q as f32, round on-device into the matmul dtype.
            # SWDGE (gpsimd) ring keeps the SP ring free for the first x load.
            p_stage = consts.tile([KC, NCH, 2 * MODES], F32)
            nc.gpsimd.dma_start(
                out=p_stage, in_=p_d.rearrange("(c k) m -> k c m", k=KC)
            )
            pP = consts.tile([KC, NCH, 2 * MODES], RND)
            nc.vector.tensor_copy(pP, p_stage)
            q_stage = consts.tile([2 * MODES, L], F32)
            nc.gpsimd.dma_start(out=q_stage, in_=q_d[:, :])
            qQ = consts.tile([2 * MODES, L], RND)
            nc.vector.tensor_copy(qQ, q_stage)

            # ragged schedule: 256-row tiles at the head and tail halve the
            # pipeline fill/drain chains; 512-row tiles in the middle.
            # 256 keeps MM1's moving dim at the f32r full-speed threshold.
            tiles = [256, 256] + [SUP] * (NSUP - 2) + [256, 256]
            row0 = 0
            for s, rows in enumerate(tiles):
                nsub = rows // 128
                x_sb = xin.tile([128, nsub, L], F32, tag="x_sb")
                # early tiles load in halves so transposes start sooner
                load_parts = (
                    [(g, g + 1) for g in range(nsub)]
                    if s <= 2
                    else [(0, nsub)]
                )
                for lo, hi in load_parts:
                    nc.sync.dma_start(
                        out=x_sb[:, lo:hi, :],
                        in_=x_d[
                            row0 + lo * 128 : row0 + hi * 128
                        ].rearrange("(j p) n -> p j n", p=128),
                    )

                # transpose 128-row sub-tiles chunk-wise into [KC, rows]
                xt_sb = xtp.tile([KC, NCH, rows], RND, tag="xt_sb")
                for j in range(nsub):
                    for c2 in range(2):
                        xt_ps = ps_xt.tile([KC, 4, 128], F32)
                        for cc in range(4):
                            c = c2 * 4 + cc
                            nc.tensor.transpose(
                                xt_ps[:, cc, :],
                                x_sb[:, j, c * KC : (c + 1) * KC],
                                ident,
                            )
                        # PSUM->SBUF rounding casts on ACT (DVE does the adds)
                        dst = xt_sb[
                            :, c2 * 4 : (c2 + 1) * 4, j * 128 : (j + 1) * 128
                        ]
                        nc.scalar.copy(dst, xt_ps[:, :, :])

                # MM1: A^T [2m, rows] accumulated over 8 chunks
                a_ps = ps_a.tile([2 * MODES, rows], F32, tag="a_ps")
                for c in range(NCH):
                    nc.tensor.matmul(
                        a_ps,
                        pP[:, c, :],
                        xt_sb[:, c, :],
                        start=(c == 0),
                        stop=(c == NCH - 1),
                    )
                a_sb = apool.tile([2 * MODES, rows], RND, tag="a_sb")
                nc.vector.tensor_copy(a_sb, a_ps)

                # MM2 + add per 128-row sub-tile
                y_sb = yout.tile([128, nsub, L], F32, tag="y_sb")
                for j in range(nsub):
                    for h in range(2):
                        corr_ps = ps_c.tile([128, HALF], F32)
                        nc.tensor.matmul(
                            corr_ps,
                            a_sb[:, j * 128 : (j + 1) * 128],
                            qQ[:, h * HALF : (h + 1) * HALF],
                            start=True,
                            stop=True,
                        )
                        nc.vector.tensor_add(
                            y_sb[:, j, h * HALF : (h + 1) * HALF],
                            x_sb[:, j, h * HALF : (h + 1) * HALF],
                            corr_ps,
                        )
                # outputs go out on the ACT HWDGE ring so a y-store waiting
                # on adds never blocks the next x-load queued on the SP ring;
                # late tiles store per sub-tile so the final drain overlaps
                # the trailing compute
                store_parts = (
                    [(g, g + 1) for g in range(nsub)]
                    if s >= len(tiles) - 3
                    else [(0, nsub)]
                )
                for lo, hi in store_parts:
                    nc.scalar.dma_start(
                        out=y_d[
                            row0 + lo * 128 : row0 + hi * 128
                        ].rearrange("(j p) n -> p j n", p=128),
                        in_=y_sb[:, lo:hi, :],
                    )
                row0 += rows

    nc.compile()
    return nc


def kernel(x, fweights, fweights_im, index):
    global _nc_cache, LAST_RESULT
    x = np.asarray(x, dtype=np.float32)
    P, Q = _build_pq(fweights, fweights_im, index)

    if _nc_cache is None:
        _nc_cache = _build_bass()
    nc = _nc_cache

    rows = np.ascontiguousarray(x.reshape(ROWS, L))
    in_maps = [
        {
            "x": rows[c * R_CORE : (c + 1) * R_CORE],
            "p": P,
            "q": Q,
        }
        for c in range(NCORES)
    ]
    res = run_bass_kernel_spmd(
        nc, in_maps, core_ids=list(range(NCORES)), trace=TRACE
    )
    LAST_RESULT = res
    y = np.concatenate([r["y"] for r in res.results], axis=0)
    return y.reshape(B, 1, E, L)

